# revision 33
# baseline (speedup 1.0000x reference)
"""Dual-pipeline LSTM kernel: two 128-example half-batches (A: free cols 0:64,
B: cols 64:128 of every group) run phase-shifted so the per-engine queues stay
dense.  Engine issue order per step is hand-interleaved:
  PE:  hA-wave, hB-wave, xA(t+1), xB(t+1)
  ACT: sifA, tgA, soA, sifB, tgB, tcA, soB, tcB
  DVE: wA, uA, addA, wB, hA, uB, addB, hB
Each pipeline keeps its own state tiles and its own PSUM banks (4 tiles x 2
banks = all 8 banks; the final FC reuses pipeline A's i-bank post-loop).
"""

from contextlib import ExitStack

import numpy as np

import concourse.bass as bass
import concourse.tile as tile
from concourse import bacc, mybir
from concourse.bass_utils import run_bass_kernel_spmd

F32 = mybir.dt.float32
BF16 = mybir.dt.bfloat16
AF = mybir.ActivationFunctionType

H, I, O = 64, 4, 4
B, T_FULL = 2048, 512
NCORES = 8
BLOC = B // NCORES
NG = 128
NH = 64                     # free-dim half (per pipeline)
KX = 2 * (1 + I)
XFOLD = 2

USE_BF16 = True
CHUNKS = (1, 0, 2, 3)       # f, i, g, o


def build_nc(T=T_FULL, use_bf16=None):
    if use_bf16 is None:
        use_bf16 = USE_BF16
    DT = BF16 if use_bf16 else F32
    assert T % XFOLD == 0
    TQ = T // XFOLD
    nc = bacc.Bacc(
        "TRN2",
        target_bir_lowering=False,
        debug=False,
        enable_asserts=False,
        num_devices=NCORES,
    )

    xq = nc.dram_tensor("xq", [128, TQ, NG], DT, kind="ExternalInput")
    wh2 = nc.dram_tensor("wh2", [2 * H, 4, 2 * H], DT, kind="ExternalInput")
    wx2 = nc.dram_tensor("wx2", [128, 4, 2 * H], DT, kind="ExternalInput")
    wfc2 = nc.dram_tensor("wfc2", [2 * H, 2 * O], DT, kind="ExternalInput")
    out = nc.dram_tensor("out", [2 * O, NG], F32, kind="ExternalOutput")

    with tile.TileContext(nc) as tc, ExitStack() as ctx:
        persist = ctx.enter_context(tc.tile_pool(name="persist", bufs=1))
        acts = ctx.enter_context(tc.tile_pool(name="acts", bufs=3))
        temps = ctx.enter_context(tc.tile_pool(name="temps", bufs=3))
        psum = ctx.enter_context(tc.tile_pool(name="psum", bufs=1, space="PSUM"))

        xall = persist.tile([128, TQ, NG], DT, tag="xall")
        nc.sync.dma_start(xall[:], xq[:])
        wh2_sb = persist.tile([2 * H, 4, 2 * H], DT, tag="wh2")
        nc.sync.dma_start(wh2_sb[:], wh2[:])
        wx2_sb = persist.tile([128, 4, 2 * H], DT, tag="wx2")
        nc.scalar.dma_start(wx2_sb[:], wx2[:])
        wfc2_sb = persist.tile([2 * H, 2 * O], DT, tag="wfc2")
        nc.scalar.dma_start(wfc2_sb[:], wfc2[:])

        BK = 512
        # Per-pipeline state
        # cst[p] is a combined tile: c state @cols 0:NH, tanh(g) dest @NH:2NH
        cst, hb, psIF, psGO = {}, {}, {}, {}
        for p in ("A", "B"):
            hb[p] = []
            for j in range(2):
                h_t = persist.tile([2 * H, NH], DT, tag=f"h{p}{j}")
                nc.vector.memset(h_t[:], 0.0)
                hb[p].append(h_t)
            ps1 = psum.tile([2 * H, BK], F32, tag=f"psIF{p}")
            psIF[p] = ps1
            ps2 = psum.tile([2 * H, BK], F32, tag=f"psGO{p}")
            psGO[p] = ps2
            ct_t = persist.tile([2 * H, 2 * NH], DT, tag=f"ct{p}")
            nc.vector.memset(ct_t[:], 0.0)
            cst[p] = ct_t

        # gate regions: f/i/o packed in psIF's single bank, g in psGO
        def regions(p):
            return {
                1: psIF[p][:, 0:NH],              # f
                0: psIF[p][:, NH : 2 * NH],       # i
                3: psIF[p][:, 2 * NH : 3 * NH],   # o
                2: psGO[p][:, 0:NH],              # g
            }

        def xs(t, p):
            q = 64 * (t % XFOLD)
            lo = 0 if p == "A" else NH
            return xall[q : q + KX, t // XFOLD, lo : lo + NH]

        def x_wave(t, p):
            q = 64 * (t % XFOLD)
            reg = regions(p)
            for ch, st in ((1, True), (0, False), (3, False), (2, True)):
                nc.tensor.matmul(
                    reg[ch], wx2_sb[q : q + KX, ch, :], xs(t, p),
                    start=st, stop=False, skip_group_check=True,
                )

        def h_wave(t, p):
            reg = regions(p)
            for ch in (1, 0, 3, 2):
                nc.tensor.matmul(
                    reg[ch], wh2_sb[:, ch, :], hb[p][t % 2][:],
                    start=False, stop=True, skip_group_check=True,
                )

        # Prologue: x-parts for step 0, both pipelines.
        x_wave(0, "A")
        x_wave(0, "B")

        for t in range(T):
            # --- PE ---
            h_wave(t, "A")
            h_wave(t, "B")

            # --- ACT / DVE interleaved (program order per engine matters) ---
            sif_, tcs_, wu_ = {}, {}, {}
            for p in ("A", "B"):
                sif_t = acts.tile([2 * H, 3 * NH], DT, tag=f"sif{p}")
                sif_[p] = sif_t
                tcs_t = acts.tile([2 * H, NH], DT, tag=f"tc{p}")
                tcs_[p] = tcs_t
                wu_t = temps.tile([2 * H, 2 * NH], DT, tag=f"wu{p}")
                wu_[p] = wu_t

            def act_sifo(p):
                # one contiguous sigmoid over [f|i|o] -> [sf|si|so]
                nc.scalar.activation(
                    sif_[p][:], psIF[p][:, 0 : 3 * NH], AF.Sigmoid
                )

            def act_tg(p):
                # tanh(g) lands next to the c state for the fused w/u multiply
                nc.scalar.activation(
                    cst[p][:, NH : 2 * NH], regions(p)[2], AF.Tanh
                )

            def act_tc(p):
                nc.scalar.activation(tcs_[p][:], cst[p][:, 0:NH], AF.Tanh)

            def dve_wu(p):
                # [sf|si] * [c|tg] -> [w|u] in one FD=128 op
                nc.vector.tensor_mul(
                    wu_[p][:], sif_[p][:, 0 : 2 * NH], cst[p][:]
                )

            def dve_add(p):
                # Pipeline B's add runs on GPSIMD: it sits on the pair's
                # critical path only via the DVE queue (hA always loses the
                # ready-race to wuB/addB); moving it off DVE lets hA start
                # at its dependency-ready time.
                eng = nc.gpsimd if p == "B" else nc.vector
                eng.tensor_add(
                    cst[p][:, 0:NH], wu_[p][:, 0:NH], wu_[p][:, NH : 2 * NH]
                )

            def dve_h(p):
                nc.vector.tensor_mul(
                    hb[p][(t + 1) % 2][:], sif_[p][:, 2 * NH : 3 * NH], tcs_[p][:]
                )

            # ACT order: sifoA tgA sifoB tgB tcA tcB
            # DVE order: wuA addA hA wuB addB hB
            act_sifo("A")
            act_tg("A")
            dve_wu("A")
            act_sifo("B")
            dve_add("A")
            act_tg("B")
            act_tc("A")
            dve_wu("B")
            dve_h("A")
            dve_add("B")
            act_tc("B")
            dve_h("B")
            # x-parts for t+1 issued last (group checker wants the reads of
            # step t's banks issued before the next group opens); PE queue
            # order is unaffected.
            if t + 1 < T:
                x_wave(t + 1, "A")
                x_wave(t + 1, "B")

        # FC tail: reuse pipeline A's i-bank for the PSUM output.
        fcA = psIF["A"][0 : 2 * O, 0:NH]
        fcB = psIF["A"][0 : 2 * O, NH:NG]
        nc.tensor.matmul(fcA, wfc2_sb[:], hb["A"][T % 2][:], start=True, stop=True)
        nc.tensor.matmul(
            fcB, wfc2_sb[:], hb["B"][T % 2][:],
            start=False, stop=True, skip_group_check=True,
        )
        fc_sb = temps.tile([2 * O, NG], F32, tag="fcsb")
        nc.vector.tensor_copy(fc_sb[:], psIF["A"][0 : 2 * O, 0:NG])
        nc.sync.dma_start(out[:], fc_sb[:])

    nc.compile()
    return nc


def prep_weights(W_ih, W_hh, b_ih, b_hh, W_fc, b_fc):
    bsum = (b_ih + b_hh).astype(np.float32)
    wh2 = np.zeros((2 * H, 4, 2 * H), np.float32)
    wx2 = np.zeros((KX, 4, 2 * H), np.float32)
    for ch in range(4):
        r = slice(ch * H, (ch + 1) * H)
        wh2[0:H, ch, 0:H] = W_hh[r].T
        wh2[H:, ch, H:] = W_hh[r].T
        wx2[0, ch, 0:H] = bsum[r]
        wx2[1 : 1 + I, ch, 0:H] = W_ih[r].T
        wx2[1 + I, ch, H:] = bsum[r]
        wx2[2 + I :, ch, H:] = W_ih[r].T
    wfc2 = np.zeros((2 * H, 2 * O), np.float32)
    wfc2[0:H, 0:O] = W_fc.T
    wfc2[H:, O:] = W_fc.T
    wx4 = np.zeros((128, 4, 2 * H), np.float32)
    for qq in range(XFOLD):
        wx4[64 * qq : 64 * qq + KX] = wx2
    return wh2, wx4, wfc2


def make_in_maps(x, W_ih, W_hh, b_ih, b_hh, W_fc, b_fc, T=T_FULL, use_bf16=None):
    import ml_dtypes

    if use_bf16 is None:
        use_bf16 = USE_BF16
    npdt = ml_dtypes.bfloat16 if use_bf16 else np.float32
    wh2, wx4, wfc2 = prep_weights(W_ih, W_hh, b_ih, b_hh, W_fc, b_fc)
    wh2, wx4, wfc2 = (a.astype(npdt) for a in (wh2, wx4, wfc2))
    TQ = T // XFOLD
    in_maps = []
    for core in range(NCORES):
        xc = x[core * BLOC : (core + 1) * BLOC, :T, :]
        xT = np.ascontiguousarray(xc.transpose(1, 2, 0))  # [T, I, BLOC]
        xT2 = np.empty((T, KX, NG), np.float32)
        xT2[:, 0, :] = 1.0
        xT2[:, 1 : 1 + I, :] = xT[:, :, 0:NG]
        xT2[:, 1 + I, :] = 1.0
        xT2[:, 2 + I :, :] = xT[:, :, NG : 2 * NG]
        xqa = np.zeros((128, TQ, NG), np.float32)
        folded = xT2.reshape(TQ, XFOLD, KX, NG).transpose(1, 2, 0, 3)
        for qq in range(XFOLD):
            xqa[64 * qq : 64 * qq + KX] = folded[qq]
        in_maps.append(
            {"xq": xqa.astype(npdt), "wh2": wh2, "wx2": wx4, "wfc2": wfc2}
        )
    return in_maps


_CACHED_NC = None


def kernel(x, W_ih, W_hh, b_ih, b_hh, W_fc, b_fc):
    global _CACHED_NC
    x = np.asarray(x, np.float32)
    args = [np.asarray(a, np.float32) for a in (W_ih, W_hh, b_ih, b_hh, W_fc, b_fc)]
    if _CACHED_NC is None:
        _CACHED_NC = build_nc()
    nc = _CACHED_NC
    in_maps = make_in_maps(x, *args)
    res = run_bass_kernel_spmd(nc, in_maps, core_ids=list(range(NCORES)))
    b_fc = args[5]
    full = np.empty((1, B, O), np.float32)
    for core in range(NCORES):
        oc = res.results[core]["out"]  # [2*O, NG]
        for g in range(2):
            lo = core * BLOC + g * NG
            full[0, lo : lo + NG, :] = oc[g * O : (g + 1) * O].T + b_fc
    return full
